# revision 15
# baseline (speedup 1.0000x reference)
"""CGCNN regressor on 8 trn2 NeuronCores — v2.

Sharding: graphs 32/core -> contiguous node blocks; edges live on dst's core.
Per core, nodes are permuted into 52 "ranges" of 128 (degree-balanced bin
packing, <=512 edges/range); each range owns 4 edge chunks of 128 slots.

Numerics: h is exchanged as fp16 hi+lo pairs (pair-table rows of 1024B =
[hi_a|hi_b|lo_a|lo_b]); conv weights are fp16 hi+lo with 3-term products
(hi*hi + hi*lo + lo*hi); edge_attr is fp16; messages and aggregation are
fp32. All activations use only the Exp/Ln act-table set (sigmoid and
softplus are computed from Exp/Ln + DVE ops) so the activation table is
loaded exactly once.

Per layer: h hi/lo staged to DRAM, AllGathered to a replicated pair table;
13 gather blocks of 2048 slots fetch h[src]; per range (512 slots) the
dst projection is computed just-in-time and messages accumulate in PSUM
[128,4,256]; aggregation is a one-hot matmul using statically preloaded
one-hot tiles. Pool/head run on 32 graphs/core; host concatenates.
"""

import os
import sys

import numpy as np

try:
    import concourse.bass as bass
except ImportError:  # grading env fallback
    sys.path.insert(0, "/opt/trn_rl_repo")
    import concourse.bass as bass

import concourse.mybir as mybir
import concourse.tile as tile
from concourse import bacc
from concourse.bass_utils import run_bass_kernel_spmd
import concourse.hw_specs as _hw_specs

# Pin every activation to the one table set that contains all functions we
# use (Exp, Ln, Abs, Copy, Identity, Relu): avoids per-instruction act-table
# reloads. Other sets are emptied (indices preserved for walrus remapping).
if not getattr(_hw_specs, "_act_tabs_pinned", False):
    _orig_gat = _hw_specs.get_activation_tables
    import functools as _ft

    @_ft.cache
    def _gat_pinned(arch):
        tabs = _orig_gat(arch)
        return {
            k: (v if k == "natural_log_exp_and_others" else set())
            for k, v in tabs.items()
        }

    _hw_specs.get_activation_tables = _gat_pinned
    bacc.get_activation_tables = _gat_pinned
    try:
        import concourse.bass_interp as _bi
        _bi.get_activation_tables = _gat_pinned
    except (ImportError, AttributeError):
        pass
    _hw_specs._act_tabs_pinned = True

F32 = np.float32
F16 = np.float16

# problem constants
N, E, H, ED, NG, NEMB, L = 50000, 200000, 128, 50, 256, 100, 6
C = 8               # cores
GPC = NG // C       # graphs per core
NT = 52             # node tiles (ranges) per core
N_LOC = NT * 128    # padded local nodes (6656)
CPR = 4             # chunks per range
NCHUNK = NT * CPR   # 208
NSLOT = NCHUNK * 128  # 26624 edge slots
CPB = 16            # chunks per gather block (4 ranges)
NBLK = NCHUNK // CPB  # 13
SLOT_B = CPB * 128  # 2048 slots per block
RPB = CPB // CPR    # 4 ranges per block
PAIRS = C * N_LOC // 2  # 26624 pair rows in the replicated h table
PAD_DST = 255.0     # dst sentinel for dummy slots (matches no one-hot row)
HSC = 1.0 / 16.0    # h-table scale (h stored as h*HSC; 16x folded into Wsrc)

_L_RUN = int(os.environ.get("KERNEL_LAYERS", str(L)))
_PHASE = int(os.environ.get("KERNEL_PHASE", "99"))  # 1=proj 2=+layers 5=all


# ---------------------------------------------------------------------------
# host-side preprocessing
# ---------------------------------------------------------------------------

def _wrap16(idx, pad_to):
    """int16 index tensor in dma_gather layout: [128, pad_to//16],
    slot i -> row i%16, col i//16; replicated 8x down the partitions."""
    a = np.full(pad_to, 0, np.int16)
    a[: len(idx)] = idx.astype(np.int16)
    w = a.reshape(pad_to // 16, 16).T  # [16, pad/16]
    return np.tile(w, (8, 1)).copy()


def _bn_fold(p, bias=None):
    gamma, beta, mean, var = [np.asarray(x, np.float64) for x in p]
    scale = gamma / np.sqrt(var + 1e-5)
    shift = beta - mean * scale
    if bias is not None:
        shift = shift + np.asarray(bias, np.float64) * scale
    return scale.astype(F32), shift.astype(F32)


def _rep(row, parts=128):
    row = np.asarray(row, F32).reshape(1, -1)
    return np.repeat(row, parts, axis=0).copy()


def _hilo(x):
    hi = np.asarray(x, F32).astype(F16)
    lo = (np.asarray(x, F32) - hi.astype(F32)).astype(F16)
    return hi, lo


def _prep(inputs):
    x_atom = np.asarray(inputs["x_atom"]).astype(np.int64)
    ei = np.asarray(inputs["edge_index"]).astype(np.int64)
    ea = np.asarray(inputs["edge_attr"]).astype(F32)
    batch = np.asarray(inputs["batch"]).astype(np.int64)
    src, dst = ei[0], ei[1]

    node_start = np.searchsorted(batch, np.arange(0, NG + 1, GPC))
    deg = np.bincount(dst, minlength=N)

    # global node -> (core, local id); degree-balanced FFD into NT ranges/core
    lid = np.empty(N, np.int64)
    core_of = np.empty(N, np.int64)
    for c in range(C):
        s, e = node_start[c], node_start[c + 1]
        nodes = np.arange(s, e)
        assert len(nodes) <= N_LOC, f"core {c}: {len(nodes)} > {N_LOC}"
        order = nodes[np.argsort(-deg[nodes], kind="stable")]
        cap_n = np.full(NT, 128, np.int64)
        cap_e = np.full(NT, CPR * 128, np.int64)
        pos = np.zeros(NT, np.int64)
        for g in order:
            d = deg[g]
            cand = np.where((cap_n > 0) & (cap_e >= d))[0]
            assert len(cand), f"core {c}: range packing failed (deg {d})"
            r = cand[np.argmax(cap_e[cand])]
            lid[g] = r * 128 + pos[r]
            pos[r] += 1
            cap_n[r] -= 1
            cap_e[r] -= d
        core_of[s:e] = c

    gaddr = core_of * N_LOC + lid  # global address in the replicated table

    in_maps = []
    for c in range(C):
        s, e = node_start[c], node_start[c + 1]
        slot_pair = np.zeros(NSLOT, np.int64)
        slot_par = np.zeros(NSLOT, F32)
        slot_dst = np.full(NSLOT, PAD_DST, F32)
        slot_ea = np.zeros((NSLOT, ED), F32)
        slot_bias = np.zeros(NSLOT, F32)

        emask = (dst >= s) & (dst < e)
        ce_src, ce_dst, ce_ea = src[emask], dst[emask], ea[emask]
        r_of_e = lid[ce_dst] // 128
        for r in range(NT):
            sel = np.where(r_of_e == r)[0]
            assert len(sel) <= CPR * 128, f"core {c} range {r}: {len(sel)}"
            base = r * CPR * 128
            sl = base + np.arange(len(sel))
            ga = gaddr[ce_src[sel]]
            slot_pair[sl] = ga >> 1
            slot_par[sl] = (ga & 1).astype(F32)
            slot_dst[sl] = (lid[ce_dst[sel]] - r * 128).astype(F32)
            slot_ea[sl] = ce_ea[sel]
            slot_bias[sl] = 1.0

        # static one-hots: ssc [slot, node] fp32 (aggr lhsT),
        # sscT [node, slot] fp16 * 16 (dst-expansion lhsT)
        dcol = slot_dst.reshape(NCHUNK, 128)  # [chunk, slot-in-chunk]
        ssc = np.zeros((128, NCHUNK * 128), F32)     # [slot_p, chunk*node]
        sscT = np.zeros((128, NCHUNK * 128), F16)    # [node_p, chunk*slot]
        for ch in range(NCHUNK):
            d_ = dcol[ch].astype(np.int64)  # dst node per slot (255 = pad)
            valid = d_ < 128
            sl_idx = np.nonzero(valid)[0]
            ssc[sl_idx, ch * 128 + d_[sl_idx]] = 1.0
            sscT[d_[sl_idx], ch * 128 + sl_idx] = np.float16(16.0)

        # graph one-hot for pooling over local (permuted) node layout
        goh = np.zeros((128, NT * GPC), F32)
        xa_local = np.zeros(N_LOC, np.int64)
        nodes = np.arange(s, e)
        li = lid[nodes]
        xa_local[li] = x_atom[nodes]
        t_i, p_i = li // 128, li % 128
        goh[p_i, t_i * GPC + (batch[nodes] - c * GPC)] = 1.0

        goh2 = np.zeros((GPC, N_LOC), F32)
        goh2[batch[nodes] - c * GPC, li] = 1.0

        m = {
            "gidx": _wrap16(slot_pair, NSLOT),
            "xidx": _wrap16(xa_local, N_LOC),
            "pmask": np.repeat(
                slot_par.reshape(1, -1), 128, axis=0
            ).astype(np.uint8),
            "eaT": np.concatenate(
                [slot_ea.T, slot_bias.reshape(1, -1)], axis=0
            ).astype(F16),
            "ssc": ssc,
            "sscT": sscT,
            "goh": goh,
            "goh2": goh2,
            "maskbias": ((goh - 1.0) * 1e30).astype(F32),
        }
        in_maps.append(m)

    # shared parameters
    conv_Wf = np.asarray(inputs["conv_Wf"], F32)
    conv_Ws = np.asarray(inputs["conv_Ws"], F32)
    conv_bf = np.asarray(inputs["conv_bf"], F32)
    conv_bs = np.asarray(inputs["conv_bs"], F32)
    conv_bn = np.asarray(inputs["conv_bn"], F32)

    wsrc = np.concatenate(
        [
            np.concatenate([conv_Wf[l, H : 2 * H], conv_Ws[l, H : 2 * H]], 1)
            for l in range(L)
        ],
        axis=1,
    ) / HSC  # [128, L*256]
    wdst = np.concatenate(
        [
            np.concatenate([conv_Wf[l, :H], conv_Ws[l, :H]], 1)
            for l in range(L)
        ],
        axis=1,
    )
    wea = np.concatenate(
        [
            np.concatenate(
                [
                    np.concatenate([conv_Wf[l, 2 * H :], conv_Ws[l, 2 * H :]], 1),
                    np.concatenate([conv_bf[l], conv_bs[l]]).reshape(1, -1),
                ],
                axis=0,
            )
            for l in range(L)
        ],
        axis=1,
    )  # [51, L*256]
    convss = np.concatenate(
        [
            np.concatenate([_rep(sc), _rep(sh)], axis=1)
            for sc, sh in ((_bn_fold(conv_bn[l])) for l in range(L))
        ],
        axis=1,
    )  # [128, L*256]

    wsrc_hi, wsrc_lo = _hilo(wsrc)
    wea_hi, wea_lo = _hilo(wea)

    psc, psh = _bn_fold(np.asarray(inputs["proj_bn"], F32),
                        bias=np.asarray(inputs["proj_b"], F32))
    h1sc, h1sh = _bn_fold(np.asarray(inputs["head_bn1"], F32),
                          bias=np.asarray(inputs["head_b1"], F32))
    h2sc, h2sh = _bn_fold(np.asarray(inputs["head_bn2"], F32),
                          bias=np.asarray(inputs["head_b2"], F32))

    shared = {
        "emb": np.asarray(inputs["emb"], F32),
        "projW": np.asarray(inputs["proj_W"], F32),
        "projss": np.concatenate([_rep(psc), _rep(psh)], axis=1),
        "wsrc_hi": wsrc_hi, "wsrc_lo": wsrc_lo,
        "wdst": wdst.astype(F32),
        "wea_hi": wea_hi, "wea_lo": wea_lo,
        "convss": convss,
        "gatew1": np.asarray(inputs["gate_W1"], F32),
        "gateb1": _rep(np.asarray(inputs["gate_b1"], F32)),
        "gatew2": np.asarray(inputs["gate_W2"], F32),
        "gateb2": _rep(np.asarray(inputs["gate_b2"], F32).reshape(1)),
        "headw1": np.asarray(inputs["head_W1"], F32),
        "h1ss": np.concatenate([_rep(h1sc), _rep(h1sh)], axis=1),
        "headw2": np.asarray(inputs["head_W2"], F32),
        "h2ss": np.concatenate([_rep(h2sc), _rep(h2sh)], axis=1),
        "headw3": np.asarray(inputs["head_W3"], F32),
        "h3b": _rep(np.asarray(inputs["head_b3"], F32)),
        "headw4": np.asarray(inputs["head_W4"], F32),
        "h4b": _rep(np.asarray(inputs["head_b4"], F32).reshape(1)),
        "iota": np.repeat(
            np.arange(128, dtype=F32).reshape(1, -1), 128, axis=0
        ),
        "identf": np.eye(128, dtype=F32),
    }
    for m in in_maps:
        m.update(shared)
    return in_maps


# ---------------------------------------------------------------------------
# bass program
# ---------------------------------------------------------------------------

def _build():
    dt = mybir.dt
    nc = bacc.Bacc(num_devices=C)

    # const AP for activation bias=30.0 (clamp-via-Relu/Exp trick)
    _c30 = nc.alloc_sbuf_tensor("const-float32-30.0", [128, 1], dt.float32)
    nc.gpsimd.memset(_c30.ap(), 30.0)
    nc.const_aps.aps[(dt.float32, 30.0)] = _c30.ap()

    def par(name, shape, dtp):
        return nc.declare_dram_parameter(name, list(shape), dtp, isOutput=False)

    gidx_d = par("gidx", [128, NSLOT // 16], dt.int16)
    xidx_d = par("xidx", [128, N_LOC // 16], dt.int16)
    pmask_d = par("pmask", [128, NSLOT], dt.uint8)
    eaT_d = par("eaT", [ED + 1, NSLOT], dt.float16)
    ssc_d = par("ssc", [128, NCHUNK * 128], dt.float32)
    sscT_d = par("sscT", [128, NCHUNK * 128], dt.float16)
    goh_d = par("goh", [128, NT * GPC], dt.float32)
    goh2_d = par("goh2", [GPC, N_LOC], dt.float32)
    maskbias_d = par("maskbias", [128, NT * GPC], dt.float32)
    emb_d = par("emb", [NEMB, H], dt.float32)
    projW_d = par("projW", [H, H], dt.float32)
    projss_d = par("projss", [128, 256], dt.float32)
    wsrc_hi_d = par("wsrc_hi", [H, L * 256], dt.float16)
    wsrc_lo_d = par("wsrc_lo", [H, L * 256], dt.float16)
    wdst_d = par("wdst", [H, L * 256], dt.float32)
    wea_hi_d = par("wea_hi", [ED + 1, L * 256], dt.float16)
    wea_lo_d = par("wea_lo", [ED + 1, L * 256], dt.float16)
    convss_d = par("convss", [128, L * 256], dt.float32)
    gatew1_d = par("gatew1", [H, H // 2], dt.float32)
    gateb1_d = par("gateb1", [128, H // 2], dt.float32)
    gatew2_d = par("gatew2", [H // 2, 1], dt.float32)
    gateb2_d = par("gateb2", [128, 1], dt.float32)
    headw1_d = par("headw1", [H, H], dt.float32)
    h1ss_d = par("h1ss", [128, 256], dt.float32)
    headw2_d = par("headw2", [H, H // 2], dt.float32)
    h2ss_d = par("h2ss", [128, 128], dt.float32)
    headw3_d = par("headw3", [H // 2, H // 4], dt.float32)
    h3b_d = par("h3b", [128, H // 4], dt.float32)
    headw4_d = par("headw4", [H // 4, 1], dt.float32)
    h4b_d = par("h4b", [128, 1], dt.float32)
    iota_d = par("iota", [128, 128], dt.float32)
    identf_d = par("identf", [128, 128], dt.float32)

    out_d = nc.declare_dram_parameter("out", [GPC, 1], dt.float32, isOutput=True)

    hstage = nc.dram_tensor("hstage", [N_LOC // 2, 512], dt.float16)
    hfull = [
        nc.dram_tensor(f"hfull{i}", [PAIRS, 512], dt.float16,
                       addr_space="Shared")
        for i in range(2)
    ]

    FT = dt.float32
    AF = mybir.ActivationFunctionType
    OP = mybir.AluOpType

    with tile.TileContext(nc) as tc:
        with (
            tc.tile_pool(name="const", bufs=1) as cpool,
            tc.tile_pool(name="state", bufs=1) as spool,
            tc.tile_pool(name="psA", bufs=2, space="PSUM") as psA,   # fs [128,4,256]
            tc.tile_pool(name="psT", bufs=2, space="PSUM") as psT,   # transposes
            tc.tile_pool(name="psD", bufs=1, space="PSUM") as psD,   # dst proj [128,256]
            tc.tile_pool(name="psG", bufs=1, space="PSUM") as psG,   # aggr [128,128]
        ):
            # ---------------- resident tiles ----------------
            def load(pool, dram, shape, dtp):
                nm = f"c_{dram.name}"
                t = pool.tile(shape, dtp, name=nm, tag=nm)
                nc.sync.dma_start(out=t[:], in_=dram[:])
                return t

            gidx_t = load(cpool, gidx_d, [128, NSLOT // 16], dt.int16)
            pmask_t = load(cpool, pmask_d, [128, NSLOT], dt.uint8)
            wsrc_hi_t = load(cpool, wsrc_hi_d, [H, L * 256], dt.float16)
            wsrc_lo_t = load(cpool, wsrc_lo_d, [H, L * 256], dt.float16)
            wdst_t = load(cpool, wdst_d, [H, L * 256], FT)
            wea_hi_t = load(cpool, wea_hi_d, [ED + 1, L * 256], dt.float16)
            wea_lo_t = load(cpool, wea_lo_d, [ED + 1, L * 256], dt.float16)
            convss_t = load(cpool, convss_d, [128, L * 256], FT)
            identf_t = load(cpool, identf_d, [128, 128], FT)

            h_loc = spool.tile([128, NT, H], FT, tag="h_loc")

            def silu_batch(wp, x_ap, out_ap, n, uniq, tagp="sl"):
                """out = x * sigmoid(x) elementwise, exp-table only.
                x_ap/out_ap: [128, n] fp32 APs (may alias tiles). `uniq`
                must be unique per call; `tagp` stable per pool+size."""
                xm = wp.tile([128, n], FT, tag=f"{tagp}_xm", name=f"{uniq}xm")
                nc.vector.tensor_scalar_min(out=xm[:], in0=x_ap, scalar1=30.0)
                ex = wp.tile([128, n], FT, tag=f"{tagp}_ex", name=f"{uniq}ex")
                nc.scalar.activation(ex[:], xm[:], AF.Exp)
                den = wp.tile([128, n], FT, tag=f"{tagp}_dn", name=f"{uniq}dn")
                nc.scalar.activation(den[:], ex[:], AF.Copy, bias=1.0)
                nc.vector.reciprocal_approx_fast(out=den[:], in_=den[:])
                nc.vector.tensor_mul(out=ex[:], in0=ex[:], in1=den[:])
                nc.vector.tensor_mul(out=out_ap, in0=x_ap, in1=ex[:])

            # ---------------- embedding + projection ----------------
            with (
                tc.tile_pool(name="proj", bufs=2) as prpool,
                tc.tile_pool(name="projc", bufs=1) as prcpool,
            ):
                xidx_t = load(prcpool, xidx_d, [128, N_LOC // 16], dt.int16)
                projW_t = load(prcpool, projW_d, [H, H], FT)
                projss_t = load(prcpool, projss_d, [128, 256], FT)
                TPG = 13  # node tiles per gather call
                for g in range(NT // TPG):
                    h0 = prpool.tile([128, TPG, H], FT, tag="h0")
                    nc.gpsimd.dma_gather(
                        h0[:], emb_d[:],
                        xidx_t[:, g * (TPG * 8) : (g + 1) * (TPG * 8)],
                        TPG * 128, TPG * 128, H, single_packet=False,
                    )
                    gbuf = prpool.tile([128, TPG, 128], FT, tag="gbuf",
                                       name=f"gbuf{g}")
                    for tt in range(TPG):
                        t = g * TPG + tt
                        pT = psT.tile([128, 128], FT, tag="tr", name=f"prT{t}")
                        nc.tensor.transpose(pT[:], h0[:, tt, :], identf_t[:])
                        hT = prpool.tile([128, 128], FT, tag="hT32",
                                         name=f"prh{t}")
                        nc.vector.tensor_copy(hT[:], pT[:])
                        pm = psD.tile([128, 256], FT, tag="pD", name=f"prm{t}")
                        nc.tensor.matmul(pm[:, :H], hT[:], projW_t[:],
                                         start=True, stop=True)
                        nc.vector.tensor_tensor(
                            out=gbuf[:, tt, :], in0=pm[:, :H],
                            in1=projss_t[:, :128], op=OP.mult)
                        nc.vector.tensor_tensor(
                            out=gbuf[:, tt, :], in0=gbuf[:, tt, :],
                            in1=projss_t[:, 128:], op=OP.add)
                    silu_batch(
                        prpool,
                        gbuf[:].rearrange("p t h -> p (t h)"),
                        h_loc[:, g * TPG : (g + 1) * TPG, :]
                        .rearrange("p t h -> p (t h)"),
                        TPG * 128, f"pj{g}", tagp="pj")

            if _PHASE <= 1:
                dbg = spool.tile([GPC, 1], FT, tag="dbg", name="dbg1")
                nc.vector.tensor_copy(dbg[:], h_loc[:GPC, 0, 0:1])
                nc.sync.dma_start(out=out_d[:], in_=dbg[:])

            # ---------------- conv layers ----------------
            with (
                tc.tile_pool(name="stg", bufs=1) as stgpool,
                tc.tile_pool(name="gbuf", bufs=2) as gpool,
                tc.tile_pool(name="sscp", bufs=1) as sscpool,
                tc.tile_pool(name="work", bufs=2) as wpool,
                tc.tile_pool(name="acts", bufs=1) as apool,
                tc.tile_pool(name="msgp", bufs=2) as mpool,
            ):
                h_hi = stgpool.tile([128, NT, 128], dt.float16, tag="h_hi")
                h_lo = stgpool.tile([128, NT, 128], dt.float16, tag="h_lo")
                for l in range(_L_RUN if _PHASE >= 2 else 0):
                    hf = hfull[l % 2]
                    lsl = slice(l * 256, (l + 1) * 256)
                    # ---- stage h as hi/lo fp16 + allgather ----
                    hv = h_loc[:].rearrange("p t h -> p (t h)")
                    nc.vector.tensor_scalar_mul(
                        out=h_hi[:].rearrange("p t h -> p (t h)"),
                        in0=hv, scalar1=HSC)
                    nc.vector.scalar_tensor_tensor(
                        out=h_lo[:].rearrange("p t h -> p (t h)"),
                        in0=hv, scalar=HSC,
                        in1=h_hi[:].rearrange("p t h -> p (t h)"),
                        op0=OP.mult, op1=OP.subtract)
                    # row layout [hi_a, lo_a, hi_b, lo_b]: node i's (hi, lo)
                    # occupy 256 contiguous cols at (i%2)*256
                    hstv = (
                        hstage[:]
                        .rearrange("n (two hl h) -> (n two) hl h", two=2, hl=2)
                        .rearrange("(t p) hl h -> p t hl h", p=128)
                    )
                    nc.sync.dma_start(out=hstv[:, :, 0, :], in_=h_hi[:])
                    nc.sync.dma_start(out=hstv[:, :, 1, :], in_=h_lo[:])
                    nc.gpsimd.collective_compute(
                        "AllGather",
                        mybir.AluOpType.bypass,
                        replica_groups=[list(range(C))],
                        ins=[hstage[:]],
                        outs=[hf[:]],
                    )

                    for b in range(NBLK):
                        bsl = slice(b * SLOT_B, (b + 1) * SLOT_B)
                        gb = gpool.tile([128, 4, SLOT_B], dt.float16, tag="gb",
                                        name=f"gb_{l}_{b}")
                        nc.gpsimd.dma_gather(
                            gb[:], hf[:],
                            gidx_t[:, b * (SLOT_B // 16) : (b + 1) * (SLOT_B // 16)],
                            SLOT_B, SLOT_B, 512, transpose=True,
                            single_packet=False,
                        )
                        # in-place pair select (row = [hi_a, lo_a, hi_b, lo_b]):
                        # m_hi = gb[:,0], m_lo = gb[:,1]
                        nc.vector.copy_predicated(gb[:, 0, :], pmask_t[:, bsl],
                                                  gb[:, 2, :])
                        nc.vector.copy_predicated(gb[:, 1, :], pmask_t[:, bsl],
                                                  gb[:, 3, :])
                        ea_t = wpool.tile([ED + 1, SLOT_B], dt.float16,
                                          tag="ea", name=f"ea_{l}_{b}")
                        nc.sync.dma_start(out=ea_t[:], in_=eaT_d[:, bsl])
                        ssc_t = sscpool.tile([128, CPB, 128], FT, tag="ssc",
                                             name=f"ssc_{l}_{b}")
                        nc.sync.dma_start(
                            out=ssc_t[:].rearrange("p c n -> p (c n)"),
                            in_=ssc_d[:, b * SLOT_B : (b + 1) * SLOT_B])
                        sscT_t = sscpool.tile([128, CPB, 128], dt.float16,
                                              tag="sscT", name=f"sT_{l}_{b}")
                        nc.sync.dma_start(
                            out=sscT_t[:].rearrange("p c n -> p (c n)"),
                            in_=sscT_d[:, b * SLOT_B : (b + 1) * SLOT_B])

                        aggrb = wpool.tile([128, RPB, 128], FT, tag="aggrb",
                                           name=f"ab_{l}_{b}")

                        for rj in range(RPB):
                            r = RPB * b + rj
                            # ---- dst projection (JIT for this range) ----
                            pT = psT.tile([128, 128], FT, tag="tr",
                                          name=f"pT_{l}_{r}")
                            nc.tensor.transpose(pT[:], h_loc[:, r, :],
                                                identf_t[:])
                            hT = wpool.tile([128, 128], FT, tag="hT",
                                            name=f"hT_{l}_{r}")
                            nc.vector.tensor_copy(hT[:], pT[:])
                            pd = psD.tile([128, 256], FT, tag="pD",
                                          name=f"pd_{l}_{r}")
                            nc.tensor.matmul(pd[:], hT[:], wdst_t[:, lsl],
                                             start=True, stop=True)
                            p_hi = wpool.tile([128, 256], dt.float16,
                                              tag="p_hi", name=f"ph_{l}_{r}")
                            nc.scalar.activation(p_hi[:], pd[:], AF.Copy,
                                                 scale=HSC)
                            p_lo = wpool.tile([128, 256], dt.float16,
                                              tag="p_lo", name=f"pl_{l}_{r}")
                            nc.vector.scalar_tensor_tensor(
                                out=p_lo[:], in0=pd[:], scalar=HSC,
                                in1=p_hi[:], op0=OP.mult, op1=OP.subtract)

                            # ---- message pre-activations fs [128,4,256] ----
                            fs = psA.tile([128, CPR, 256], FT, tag="fs",
                                          name=f"fs_{l}_{r}")
                            for j in range(CPR):
                                cj = rj * CPR + j  # chunk within block
                                sl = slice(cj * 128, (cj + 1) * 128)
                                fj = fs[:, j, :]
                                nc.tensor.matmul(
                                    fj, gb[:, 0, sl], wsrc_hi_t[:, lsl],
                                    start=True, stop=False)
                                nc.tensor.matmul(
                                    fj, gb[:, 0, sl], wsrc_lo_t[:, lsl],
                                    start=False, stop=False)
                                nc.tensor.matmul(
                                    fj, gb[:, 1, sl], wsrc_hi_t[:, lsl],
                                    start=False, stop=False)
                                nc.tensor.matmul(
                                    fj, ea_t[:, sl], wea_hi_t[:, lsl],
                                    start=False, stop=False)
                                nc.tensor.matmul(
                                    fj, ea_t[:, sl], wea_lo_t[:, lsl],
                                    start=False, stop=False)
                                nc.tensor.matmul(
                                    fj, sscT_t[:, cj, :], p_hi[:],
                                    start=False, stop=False)
                                nc.tensor.matmul(
                                    fj, sscT_t[:, cj, :], p_lo[:],
                                    start=False, stop=True)

                            # ---- activations: msg = sig(f)*softplus(s) ----
                            f_ap = fs[:, :, 0:128]
                            s_ap = fs[:, :, 128:256]
                            sh3 = [128, CPR, 128]
                            fc = apool.tile(sh3, FT, tag="fc",
                                            name=f"fc_{l}_{r}")
                            nc.scalar.activation(fc[:], f_ap, AF.Relu,
                                                 scale=-1.0, bias=30.0)
                            ef = apool.tile(sh3, FT, tag="ef",
                                            name=f"ef_{l}_{r}")
                            nc.scalar.activation(ef[:], fc[:], AF.Exp,
                                                 scale=-1.0, bias=30.0)
                            den = apool.tile(sh3, FT, tag="den",
                                             name=f"dn_{l}_{r}")
                            nc.scalar.activation(den[:], ef[:], AF.Copy,
                                                 bias=1.0)
                            nc.vector.reciprocal_approx_fast(out=den[:],
                                                             in_=den[:])
                            nc.vector.tensor_mul(out=ef[:], in0=ef[:],
                                                 in1=den[:])  # ef = sig(f)
                            u2 = apool.tile(sh3, FT, tag="u2",
                                            name=f"u2_{l}_{r}")
                            nc.scalar.activation(u2[:], s_ap, AF.Abs)
                            nc.scalar.activation(u2[:], u2[:], AF.Exp,
                                                 scale=-1.0)  # e^-|s|
                            lnt = apool.tile(sh3, FT, tag="lnt",
                                             name=f"ln_{l}_{r}")
                            nc.scalar.activation(lnt[:], u2[:], AF.Ln,
                                                 bias=1.0)  # ln(1+e^-|s|)
                            sp = apool.tile(sh3, FT, tag="sp",
                                            name=f"sp_{l}_{r}")
                            nc.vector.scalar_tensor_tensor(
                                out=sp[:], in0=s_ap, scalar=0.0,
                                in1=lnt[:], op0=OP.max, op1=OP.add)
                            msg = mpool.tile(sh3, FT, tag="msg",
                                             name=f"ms_{l}_{r}")
                            nc.vector.tensor_mul(out=msg[:], in0=ef[:],
                                                 in1=sp[:])

                            # ---- aggregate into [node, h] ----
                            ag = psG.tile([128, 128], FT, tag="aggr",
                                          name=f"ag_{l}_{r}")
                            for j in range(CPR):
                                cj = rj * CPR + j
                                nc.tensor.matmul(
                                    ag[:], ssc_t[:, cj, :], msg[:, j, :],
                                    start=(j == 0), stop=(j == CPR - 1))
                            nc.scalar.activation(aggrb[:, rj, :], ag[:],
                                                 AF.Copy)

                        # ---- batched node update for the block's 4 ranges --
                        hb = h_loc[:, RPB * b : RPB * (b + 1), :].rearrange(
                            "p t h -> p (t h)")
                        ab = aggrb[:].rearrange("p t h -> p (t h)")
                        ub = wpool.tile([128, RPB * 128], FT, tag="ub",
                                        name=f"ub_{l}_{b}")
                        nc.vector.tensor_tensor(out=ub[:], in0=ab, in1=hb,
                                                op=OP.add)
                        ssl = convss_t[:, lsl]
                        for rj in range(RPB):
                            seg = slice(rj * 128, (rj + 1) * 128)
                            nc.vector.tensor_tensor(
                                out=ub[:, seg], in0=ub[:, seg],
                                in1=ssl[:, :128], op=OP.mult)
                            nc.vector.tensor_tensor(
                                out=ub[:, seg], in0=ub[:, seg],
                                in1=ssl[:, 128:], op=OP.add)
                        nw_u = RPB * 128
                        uxm = wpool.tile([128, nw_u], FT, tag="up_xm",
                                         name=f"uxm{l}_{b}")
                        nc.scalar.activation(uxm[:], ub[:], AF.Relu,
                                             scale=-1.0, bias=30.0)
                        uex = wpool.tile([128, nw_u], FT, tag="up_ex",
                                         name=f"uex{l}_{b}")
                        nc.scalar.activation(uex[:], uxm[:], AF.Exp,
                                             scale=-1.0, bias=30.0)
                        udn = wpool.tile([128, nw_u], FT, tag="up_dn",
                                         name=f"udn{l}_{b}")
                        nc.scalar.activation(udn[:], uex[:], AF.Copy, bias=1.0)
                        nc.vector.reciprocal_approx_fast(out=udn[:],
                                                         in_=udn[:])
                        nc.vector.tensor_mul(out=uex[:], in0=uex[:],
                                             in1=udn[:])
                        nc.vector.tensor_mul(out=uxm[:], in0=ub[:],
                                             in1=uex[:])
                        nc.vector.tensor_tensor(out=hb, in0=hb, in1=uxm[:],
                                                op=OP.add)

            if _PHASE in (2, 3, 4):
                dbg2 = spool.tile([GPC, 1], FT, tag="dbg", name="dbg2")
                nc.vector.tensor_copy(dbg2[:], h_loc[:GPC, 0, 0:1])
                nc.sync.dma_start(out=out_d[:], in_=dbg2[:])

            # ---------------- gate + pooling + head ----------------
            with (
                tc.tile_pool(name="poolc", bufs=1) as pcpool,
                tc.tile_pool(name="pools", bufs=3) as smpool,
            ):
              if _PHASE >= 5:
                goh_t = load(pcpool, goh_d, [128, NT * GPC], FT)
                goh2_t = load(pcpool, goh2_d, [GPC, N_LOC], FT)
                maskb_t = load(pcpool, maskbias_d, [128, NT * GPC], FT)
                gatew1_t = load(pcpool, gatew1_d, [H, H // 2], FT)
                gateb1_t = load(pcpool, gateb1_d, [128, H // 2], FT)
                gatew2_t = load(pcpool, gatew2_d, [H // 2, 1], FT)
                gateb2_t = load(pcpool, gateb2_d, [128, 1], FT)
                headw1_t = load(pcpool, headw1_d, [H, H], FT)
                h1ss_t = load(pcpool, h1ss_d, [128, 256], FT)
                headw2_t = load(pcpool, headw2_d, [H, H // 2], FT)
                h2ss_t = load(pcpool, h2ss_d, [128, 128], FT)
                headw3_t = load(pcpool, headw3_d, [H // 2, H // 4], FT)
                h3b_t = load(pcpool, h3b_d, [128, H // 4], FT)
                headw4_t = load(pcpool, headw4_d, [H // 4, 1], FT)
                h4b_t = load(pcpool, h4b_d, [128, 1], FT)

                g_all = pcpool.tile([128, NT], FT, name="g_all", tag="g_all")
                runmax = pcpool.tile([128, GPC], FT, name="runmax",
                                     tag="runmax")
                s1buf = pcpool.tile([128, NT, H // 2], FT, name="s1buf",
                                    tag="s1buf")

                # pass 1a: s1 pre-activations for all tiles
                for t in range(NT):
                    pT = psT.tile([128, 128], FT, tag="tr", name=f"gT{t}")
                    nc.tensor.transpose(pT[:], h_loc[:, t, :], identf_t[:])
                    hT = smpool.tile([128, 128], FT, tag="ghT", name=f"ghT{t}")
                    nc.vector.tensor_copy(hT[:], pT[:])
                    g1 = psD.tile([128, 256], FT, tag="pD", name=f"g1_{t}")
                    nc.tensor.matmul(g1[:, : H // 2], hT[:],
                                     gatew1_t[:], start=True, stop=True)
                    nc.vector.tensor_tensor(
                        out=s1buf[:, t, :], in0=g1[:, : H // 2],
                        in1=gateb1_t[:], op=OP.add)
                # batched silu, 13 tiles per group
                for gg in range(NT // 13):
                    sl_g = s1buf[:, gg * 13 : (gg + 1) * 13, :].rearrange(
                        "p t h -> p (t h)")
                    silu_batch(smpool, sl_g, sl_g, 13 * (H // 2),
                               f"gs{gg}", tagp="gs")
                # pass 1b: g scores + masked running max
                for t in range(NT):
                    pT2 = psT.tile([128, 128], FT, tag="tr", name=f"gU{t}")
                    nc.tensor.transpose(pT2[: H // 2, :], s1buf[:, t, :],
                                        identf_t[:])
                    s1T = smpool.tile([H // 2, 128], FT, tag="s1T",
                                      name=f"s1T_{t}")
                    nc.vector.tensor_copy(s1T[:], pT2[: H // 2, :])
                    g2 = psT.tile([128, 128], FT, tag="tr", name=f"g2_{t}")
                    nc.tensor.matmul(g2[:, :1], s1T[:], gatew2_t[:],
                                     start=True, stop=True)
                    nc.vector.tensor_tensor(
                        out=g_all[:, t : t + 1], in0=g2[:, :1],
                        in1=gateb2_t[:], op=OP.add)
                    gm = smpool.tile([128, GPC], FT, tag="gm",
                                     name=f"gm_{t}")
                    nc.vector.tensor_tensor(
                        out=gm[:],
                        in0=g_all[:, t : t + 1].to_broadcast([128, GPC]),
                        in1=goh_t[:, t * GPC : (t + 1) * GPC], op=OP.mult)
                    nc.vector.tensor_tensor(
                        out=gm[:], in0=gm[:],
                        in1=maskb_t[:, t * GPC : (t + 1) * GPC], op=OP.add)
                    if t == 0:
                        nc.vector.tensor_copy(runmax[:], gm[:])
                    else:
                        nc.vector.tensor_max(out=runmax[:], in0=runmax[:],
                                             in1=gm[:])

                # reduce running max across partitions -> -gmax [GPC, 1]
                pTm = psT.tile([128, 128], FT, tag="tr", name="pTm")
                nc.tensor.transpose(pTm[:GPC, :], runmax[:], identf_t[:])
                rmT = smpool.tile([GPC, 128], FT, tag="rmT", name="rmT")
                nc.vector.tensor_copy(rmT[:], pTm[:GPC, :])
                negmax = smpool.tile([GPC, 1], FT, tag="negmax",
                                     name="negmax")
                nc.vector.tensor_reduce(out=negmax[:], in_=rmT[:],
                                        axis=mybir.AxisListType.X,
                                        op=OP.max)
                nc.vector.tensor_scalar_mul(out=negmax[:], in0=negmax[:],
                                            scalar1=-1.0)

                # pass 2: e = exp(min(g - gmax[graph], 20)), pooled sums
                nKb = pcpool.tile([128, NT], FT, name="nKb", tag="nKb")
                for t in range(NT):
                    nK = psT.tile([128, 128], FT, tag="tr", name=f"nK{t}")
                    nc.tensor.matmul(
                        nK[:, :1], goh2_t[:, t * 128 : (t + 1) * 128],
                        negmax[:], start=True, stop=True)
                    nc.vector.tensor_copy(nKb[:, t : t + 1], nK[:, :1])
                earg = pcpool.tile([128, NT], FT, name="earg", tag="earg")
                nc.vector.tensor_tensor(out=earg[:], in0=g_all[:],
                                        in1=nKb[:], op=OP.add)
                nc.vector.tensor_scalar_min(out=earg[:], in0=earg[:],
                                            scalar1=20.0)
                nc.scalar.activation(earg[:], earg[:], AF.Exp)

                pool_ps = psA.tile([GPC, H + 1], FT, tag="fs", name="pool_ps")
                for t in range(NT):
                    rhs = smpool.tile([128, H + 1], FT, tag="rhs",
                                      name=f"rhs_{t}")
                    nc.vector.tensor_scalar(
                        out=rhs[:, :H], in0=h_loc[:, t, :],
                        scalar1=earg[:, t : t + 1], scalar2=None, op0=OP.mult)
                    nc.vector.tensor_copy(rhs[:, H : H + 1],
                                          earg[:, t : t + 1])
                    nc.tensor.matmul(
                        pool_ps[:], goh_t[:, t * GPC : (t + 1) * GPC], rhs[:],
                        start=(t == 0), stop=(t == NT - 1))

                pooled_raw = smpool.tile([GPC, H + 1], FT, tag="praw")
                nc.vector.tensor_copy(pooled_raw[:], pool_ps[:])
                rec = smpool.tile([GPC, 1], FT, tag="rec")
                nc.vector.reciprocal(rec[:], pooled_raw[:, H : H + 1])
                pooled = smpool.tile([GPC, H], FT, tag="pooled")
                nc.vector.tensor_scalar(
                    out=pooled[:], in0=pooled_raw[:, :H], scalar1=rec[:],
                    scalar2=None, op0=OP.mult)

                def head_silu(y, nout, nm):
                    ysg = smpool.tile([GPC, nout], FT, tag=f"hsg{nout}",
                                      name=f"ysg{nm}")
                    nc.vector.tensor_scalar_min(out=ysg[:], in0=y[:],
                                                scalar1=30.0)
                    nc.scalar.activation(ysg[:], ysg[:], AF.Exp)
                    dn = smpool.tile([GPC, nout], FT, tag=f"hdn{nout}",
                                     name=f"ydn{nm}")
                    nc.scalar.activation(dn[:], ysg[:], AF.Copy, bias=1.0)
                    nc.vector.reciprocal_approx_fast(out=dn[:], in_=dn[:])
                    nc.vector.tensor_mul(out=ysg[:], in0=ysg[:], in1=dn[:])
                    nc.vector.tensor_mul(out=y[:], in0=y[:], in1=ysg[:])

                def head_mm(x, w, nin, nout, nm, ss=None, badd=None,
                            silu=True):
                    pT = psT.tile([128, 128], FT, tag="tr",
                                  name=f"hT{nm}")
                    nc.tensor.transpose(pT[:nin, :GPC], x[:],
                                        identf_t[:GPC, :GPC])
                    xT = smpool.tile([128, GPC], FT, tag="xT",
                                     name=f"xT{nm}")
                    nc.vector.tensor_copy(xT[:nin, :], pT[:nin, :GPC])
                    ym = psD.tile([128, 256], FT, tag="pD", name=f"ym{nm}")
                    nc.tensor.matmul(ym[:GPC, :nout], xT[:nin, :], w[:],
                                     start=True, stop=True)
                    y = smpool.tile([GPC, nout], FT, tag=f"hd{nout}",
                                    name=f"y{nm}")
                    if ss is not None:
                        nc.vector.tensor_tensor(
                            out=y[:], in0=ym[:GPC, :nout],
                            in1=ss[:GPC, :nout], op=OP.mult)
                        nc.vector.tensor_tensor(
                            out=y[:], in0=y[:], in1=ss[:GPC, nout : 2 * nout],
                            op=OP.add)
                    elif badd is not None:
                        nc.vector.tensor_tensor(
                            out=y[:], in0=ym[:GPC, :nout],
                            in1=badd[:GPC, :nout], op=OP.add)
                    else:
                        nc.vector.tensor_copy(y[:], ym[:GPC, :nout])
                    if silu:
                        head_silu(y, nout, nm)
                    return y

                y1 = head_mm(pooled, headw1_t, H, H, "a", ss=h1ss_t)
                y2 = head_mm(y1, headw2_t, H, H // 2, "b", ss=h2ss_t)
                y3 = head_mm(y2, headw3_t, H // 2, H // 4, "c", badd=h3b_t)
                y4 = head_mm(y3, headw4_t, H // 4, 1, "d", badd=h4b_t,
                             silu=False)
                nc.sync.dma_start(out=out_d[:], in_=y4[:])

    return nc


_NC_CACHE = None
_LAST_EXEC_NS = None


def kernel(**inputs) -> np.ndarray:
    global _NC_CACHE, _LAST_EXEC_NS
    in_maps = _prep(inputs)
    if _NC_CACHE is None:
        _NC_CACHE = _build()
        _NC_CACHE.finalize()
    trace = os.environ.get("KERNEL_TRACE", "0") == "1"
    res = run_bass_kernel_spmd(
        _NC_CACHE, in_maps, core_ids=list(range(C)), trace=trace
    )
    _LAST_EXEC_NS = res.exec_time_ns
    out = np.concatenate(
        [np.asarray(res.results[c]["out"]).reshape(GPC) for c in range(C)]
    )
    return out.astype(F32)


if __name__ == "__main__":
    import jax

    with jax.default_device(jax.devices("cpu")[0]):
        sys.path.insert(0, os.path.dirname(os.path.abspath(__file__)))
        import reference

        inp = {k: np.asarray(v) for k, v in reference.setup_inputs().items()}
    y = kernel(**inp)
    print("out[:8]:", y[:8])
